# revision 22
# baseline (speedup 1.0000x reference)
"""TRN2 Bass kernel for nn_CrossModalAttention_75316546503126.

Mathematical collapse (verified against the jax reference): the acoustic
features are broadcast across the sequence axis before the K/V projections,
so every K/V row is identical, every attention row sees a constant score
vector, softmax is exactly uniform (S = 2048 = 2^11, 1/S exact in fp32), and

    attn_out[b, s, :] = v_b   with  v_b = (ac_b @ Wa + ba) @ Wv + bv
    out[b, s, :]      = x[b, s, :] @ Wt + (bt + v_b)

i.e. one [S, D] @ [D, D] matmul per batch plus a per-batch bias row.

Sharding: data-parallel over batch B=8, one batch per NeuronCore.

Layout/precision (host does only sharding/layout/weight-folding prep):
  * x pre-transposed per batch on the host, cast fp16 -> PE stationary
    operand directly, no on-chip transposes.
  * Wt cast fp16, moving operand in natural [k, n] layout; fp32 PSUM accum.
  * Weight-only folds Wa' = Wa @ Wv, c = bt + ba @ Wv + bv collapse the
    bias path to [ac|1] @ [[Wa'];[c]], computed on-PE via a column-broadcast
    stationary; acx and wac are packed in ONE [17, 769] fp16 dram tensor so
    a single small DMA (first on the SP ring) delivers both.
  * Output written fp16, upcast on host.

Schedule (engine queues), tuned against NTFF traces:
  PE    : 48 N=128 warm-up matmuls on a memset tile (fills the ~6us DMA
          prologue, gets the HAM clock-gate to 2.4 GHz before real work;
          any PE-idle gap resets the warm-up window) -> 2 bias matmuls ->
          chunk0 k-major over 4 tiles with zero-bridge matmuls (0*0
          accumulated into the open group) spanning feed micro-stalls ->
          chunks 1-3 per-tile k-loops.
  DVE   : bias copy -> per-tile psum+bias evictions to fp16.
  ACT   : W in 3 DMAs (k0 | k12 | k345) sized so FIFO completions match the
          PE's k-round cadence, then output stores.
  SP    : wacx first (26KB, its sem gates the bias matmuls), x0 in 3 DMAs,
          then chunks 1-3 (chunk 1 halved); final 256-col store.
  GPSIMD: warm-tile memset only.
PSUM: 4 tags x [128, 768] fp32 = 8 banks; warm/bias rotate through tag p3.
Measured ~51.5us vs the 82.4us prior baseline (PE streaming floor 30.7us,
NEFF preamble ~6.7us, first-DMA completion ~13us, tail ~5.8us).
"""
import sys

if "/opt/trn_rl_repo" not in sys.path:
    sys.path.insert(0, "/opt/trn_rl_repo")

from contextlib import ExitStack

import numpy as np

import concourse.bacc as bacc
import concourse.mybir as mybir
import concourse.tile as tile
from concourse.bass_utils import run_bass_kernel_spmd

F32 = mybir.dt.float32
F32R = mybir.dt.float32r
F16 = mybir.dt.float16

B, S, D = 8, 2048, 768
KB = D // 128          # 6 contraction blocks
ST = S // 128          # 16 sequence tiles per core
CT = 4                 # sequence tiles per chunk
NCH = ST // CT         # 4 chunks
N_WARM = 48            # warm-up matmuls (N=128) during the DMA prologue
N_CORES = 8

MODE = "f16"


def build_program(mode=MODE):
    nc = bacc.Bacc()

    xt = nc.declare_dram_parameter("xt", [D, S], F16, isOutput=False)
    w = nc.declare_dram_parameter("w", [D, D], F16, isOutput=False)
    wacx = nc.declare_dram_parameter("wacx", [17, 1 + D], F16, isOutput=False)
    out = nc.declare_dram_parameter("out", [S, D], F16, isOutput=True)

    with tile.TileContext(nc) as tc, ExitStack() as ctx:
        const = ctx.enter_context(tc.tile_pool(name="const", bufs=1))
        wpool = ctx.enter_context(tc.tile_pool(name="wpool", bufs=1))
        xpool = ctx.enter_context(tc.tile_pool(name="xpool", bufs=3))
        opool = ctx.enter_context(tc.tile_pool(name="opool", bufs=2))
        pso = ctx.enter_context(tc.tile_pool(name="pso", bufs=1, space="PSUM"))

        # ---- warm tile memset on gpsimd; packed acx/wac goes FIRST on the
        # SP HWDGE ring (26KB) so its completion sem fires early -- the bias
        # matmuls head-of-line block the PE queue on it ----
        warm_sb = const.tile([128, 512], F16)
        nc.gpsimd.memset(warm_sb[:], 0.0)
        wacx_sb = const.tile([17, 1 + D], F16)
        nc.sync.dma_start(wacx_sb[:], wacx[:])
        bias_sb = const.tile([128, D], F32)

        # ---- prologue: 3 DMAs per HWDGE ring, sized so completions (FIFO
        # per ring) land just before each k-round needs them.  Ring A (SP)
        # carries all of x0 and finishes early, which hands ring B (ACT,
        # all of W) the full SDMA bandwidth for its tail.  7 total prologue
        # DMAs stay within the 8 completion-sem lanes (no issue blocking).
        wsb = wpool.tile([128, KB * D], F16)
        x_chunks = {}
        xs0 = xpool.tile([128, KB * CT * 128], F16, tag="xc", name="xc0")
        x_chunks[0] = xs0
        CW = CT * 128

        def _x0dma(k0, k1):
            if k1 - k0 == 1:
                nc.sync.dma_start(xs0[:, k0 * CW:k1 * CW],
                                  xt[k0 * 128:k1 * 128, 0:CW])
            else:
                nc.sync.dma_start(
                    xs0[:, k0 * CW:k1 * CW].rearrange(
                        "p (k s) -> p k s", k=k1 - k0),
                    xt[k0 * 128:k1 * 128, 0:CW].rearrange(
                        "(k p) s -> p k s", p=128))

        def _wdma(k0, k1):
            if k1 - k0 == 1:
                nc.scalar.dma_start(wsb[:, k0 * D:k1 * D],
                                    w[k0 * 128:k1 * 128, :])
            else:
                nc.scalar.dma_start(
                    wsb[:, k0 * D:k1 * D].rearrange(
                        "p (k n) -> p k n", k=k1 - k0),
                    w[k0 * 128:k1 * 128, :].rearrange(
                        "(k p) n -> p k n", p=128))

        _x0dma(0, 1)
        _wdma(0, 1)
        _x0dma(1, 3)
        _wdma(1, 3)
        _x0dma(3, 6)
        _wdma(3, 6)

        # ---- later x chunks on the SP ring (chunk 1 in halves) ----
        def xdma(c, halves=False):
            xs = xpool.tile([128, KB * CT * 128], F16, tag="xc", name=f"xc{c}")
            cs = slice(c * CT * 128, (c + 1) * CT * 128)
            if halves:
                half = CT * 128 // 2
                xs3d = xs[:].rearrange("p (k s) -> p k s", k=KB)
                for h in range(2):
                    hs = slice(c * CT * 128 + h * half,
                               c * CT * 128 + (h + 1) * half)
                    nc.sync.dma_start(
                        xs3d[:, :, h * half:(h + 1) * half],
                        xt[:, hs].rearrange("(k p) s -> p k s", p=128))
            else:
                nc.sync.dma_start(
                    xs[:].rearrange("p (k s) -> p k s", k=KB),
                    xt[:, cs].rearrange("(k p) s -> p k s", p=128))
            x_chunks[c] = xs

        xdma(1, halves=True)
        xdma(2)
        xdma(3)

        # ---- PE: warm-up matmuls into the p3 psum buffer ----
        warm_ps = pso.tile([128, D], F32, tag="p3", name="warm_ps")
        for i in range(N_WARM):
            nc.tensor.matmul(warm_ps[:, 0:128], warm_sb[:, 0:128],
                             warm_sb[:, 0:128], start=True, stop=True,
                             skip_group_check=True)

        # ---- PE: bias row = broadcast([ac|1]) @ [[Wa@Wv];[bt+ba@Wv+bv]] ----
        bias_ps = pso.tile([128, D], F32, tag="p3", name="bias_ps")
        acx_bc = wacx_sb[:, 0:1].broadcast_to([17, 128])
        nc.tensor.matmul(bias_ps[:, 0:512], acx_bc, wacx_sb[:, 1:513],
                         start=True, stop=True, skip_group_check=True)
        nc.tensor.matmul(bias_ps[:, 512:768], acx_bc, wacx_sb[:, 513:769],
                         start=True, stop=True, skip_group_check=True)
        nc.vector.tensor_copy(bias_sb[:], bias_ps[:])

        # ---- main loop ----
        def tile_matmuls(ps, xs, t, k):
            st, sp = (k == 0), (k == KB - 1)
            lhsT = xs[:, k * CT * 128 + t * 128:k * CT * 128 + (t + 1) * 128]
            nc.tensor.matmul(ps[:, 0:512], lhsT, wsb[:, k * D:k * D + 512],
                             start=st, stop=sp, skip_group_check=True)
            nc.tensor.matmul(ps[:, 512:768], lhsT,
                             wsb[:, k * D + 512:(k + 1) * D],
                             start=st, stop=sp, skip_group_check=True)

        def emit_chunk(c, k_major):
            xs = x_chunks.pop(c)
            osup = opool.tile([128, CT * D], F16, tag="osup", name=f"osup{c}")
            pss = [pso.tile([128, D], F32, tag=f"p{t}", name=f"ps{c}_{t}")
                   for t in range(CT)]
            if k_major:
                # zero-bridges (0*0 accumulated into the open group): keep the
                # PE issue stream gapless across feed micro-stalls -- any PE
                # idle gap resets the HAM warm-up window back to 1.2 GHz
                bridges = {0: 18, 1: 6, 2: 2}
                for k in range(KB):
                    for t in range(CT):
                        tile_matmuls(pss[t], xs, t, k)
                    for _ in range(bridges.get(k, 0)):
                        nc.tensor.matmul(pss[0][:, 0:128], warm_sb[:, 0:128],
                                         warm_sb[:, 0:128], start=False,
                                         stop=False, skip_group_check=True)
            else:
                for t in range(CT):
                    for k in range(KB):
                        tile_matmuls(pss[t], xs, t, k)
            for t in range(CT):
                last = (c == NCH - 1) and (t == CT - 1)
                if last:
                    # split the final eviction/store so the kernel-ending
                    # store (and its receipt) covers only 256 columns
                    ob = osup[:, t * D:(t + 1) * D]
                    i = c * CT + t
                    nc.vector.tensor_add(ob[:, 0:512], pss[t][:, 0:512],
                                         bias_sb[:, 0:512])
                    nc.scalar.dma_start(out[i * 128:(i + 1) * 128, 0:512],
                                        ob[:, 0:512])
                    nc.vector.tensor_add(ob[:, 512:768], pss[t][:, 512:768],
                                         bias_sb[:, 512:768])
                    nc.sync.dma_start(out[i * 128:(i + 1) * 128, 512:768],
                                      ob[:, 512:768])
                    continue
                nc.vector.tensor_add(osup[:, t * D:(t + 1) * D],
                                     pss[t][:], bias_sb[:])
                if c == NCH - 1:
                    i = c * CT + t
                    nc.scalar.dma_start(out[i * 128:(i + 1) * 128, :],
                                        osup[:, t * D:(t + 1) * D])
            if c != NCH - 1:
                nc.scalar.dma_start(
                    out[c * CT * 128:(c + 1) * CT * 128, :].rearrange(
                        "(j p) d -> p j d", p=128),
                    osup[:].rearrange("p (j d) -> p j d", j=CT))

        emit_chunk(0, k_major=True)
        emit_chunk(1, k_major=False)
        emit_chunk(2, k_major=False)
        emit_chunk(3, k_major=False)

    nc.compile()
    return nc


_PROGRAM_CACHE = {}


def _get_program(mode=None):
    if mode is None:
        mode = MODE
    if mode not in _PROGRAM_CACHE:
        _PROGRAM_CACHE[mode] = build_program(mode)
    return _PROGRAM_CACHE[mode]


def _build_in_maps(text_features, acoustic_features, Wt, bt, Wa, ba, Wv, bv,
                   **_unused):
    text_features = np.asarray(text_features, dtype=np.float32)
    acoustic_features = np.asarray(acoustic_features, dtype=np.float32)
    Wt = np.asarray(Wt, dtype=np.float32)
    bt = np.asarray(bt, dtype=np.float32)
    Wa = np.asarray(Wa, dtype=np.float32)
    ba = np.asarray(ba, dtype=np.float32)
    Wv = np.asarray(Wv, dtype=np.float32)
    bv = np.asarray(bv, dtype=np.float32)

    x16 = text_features.astype(np.float16)             # [B, S, D]
    w16 = np.ascontiguousarray(Wt.astype(np.float16))  # [D, D] k-major
    # weight-only folds (input-independent preprocessing)
    wa_fold = Wa.astype(np.float64) @ Wv.astype(np.float64)        # [16, D]
    c_fold = ba.astype(np.float64) @ Wv.astype(np.float64) + bv + bt

    in_maps = []
    for b in range(N_CORES):
        wacx = np.zeros((17, 1 + D), np.float16)
        wacx[0:16, 0] = acoustic_features[b]
        wacx[16, 0] = 1.0
        wacx[0:16, 1:] = wa_fold
        wacx[16, 1:] = c_fold
        in_maps.append(dict(
            xt=np.ascontiguousarray(x16[b].T),         # [D, S] fp16
            w=w16,
            wacx=wacx,
        ))
    return in_maps


def kernel(text_features, acoustic_features, Wt, bt, Wa, ba, Wq, bq, Wk, bk,
           Wv, bv, **_unused):
    nc = _get_program()
    in_maps = _build_in_maps(text_features, acoustic_features, Wt, bt, Wa, ba,
                             Wv, bv)
    res = run_bass_kernel_spmd(nc, in_maps, list(range(N_CORES))).results
    out = np.empty((B, S, D), dtype=np.float32)
    for b in range(N_CORES):
        out[b] = res[b]["out"].astype(np.float32)
    return out


# revision 24
# speedup vs baseline: 1.0014x; 1.0014x over previous
"""TRN2 Bass kernel for nn_CrossModalAttention_75316546503126.

Mathematical collapse (verified against the jax reference): the acoustic
features are broadcast across the sequence axis before the K/V projections,
so every K/V row is identical, every attention row sees a constant score
vector, softmax is exactly uniform (S = 2048 = 2^11, 1/S exact in fp32), and

    attn_out[b, s, :] = v_b   with  v_b = (ac_b @ Wa + ba) @ Wv + bv
    out[b, s, :]      = x[b, s, :] @ Wt + (bt + v_b)

i.e. one [S, D] @ [D, D] matmul per batch plus a per-batch bias row.

Sharding: data-parallel over batch B=8, one batch per NeuronCore.

Layout/precision (host does only sharding/layout/weight-folding prep):
  * x pre-transposed per batch on the host, cast fp16 -> PE stationary
    operand directly, no on-chip transposes.
  * Wt cast fp16, moving operand in natural [k, n] layout; fp32 PSUM accum.
  * Weight-only folds Wa' = Wa @ Wv, c = bt + ba @ Wv + bv collapse the
    bias path to [ac|1] @ [[Wa'];[c]], computed on-PE via a column-broadcast
    stationary; acx and wac are packed in ONE [17, 769] fp16 dram tensor so
    a single small DMA (first on the SP ring) delivers both.
  * Output written fp16, upcast on host.

Schedule (engine queues), tuned against NTFF traces:
  PE    : 48 N=128 warm-up matmuls on a memset tile (fills the ~6us DMA
          prologue, gets the HAM clock-gate to 2.4 GHz before real work;
          any PE-idle gap resets the warm-up window) -> 2 bias matmuls ->
          chunk0 k-major over 4 tiles with zero-bridge matmuls (0*0
          accumulated into the open group) spanning feed micro-stalls ->
          chunks 1-3 per-tile k-loops.
  DVE   : bias copy -> per-tile psum+bias evictions to fp16.
  ACT   : W in 3 DMAs (k0 | k12 | k345) sized so FIFO completions match the
          PE's k-round cadence, then output stores.
  SP    : wacx first (26KB, its sem gates the bias matmuls), x0 in 3 DMAs,
          then chunks 1-3 (chunk 1 halved); final 256-col store.
  GPSIMD: warm-tile memset only.
PSUM: 4 tags x [128, 768] fp32 = 8 banks; warm/bias rotate through tag p3.
Measured ~51.5us vs the 82.4us prior baseline (PE streaming floor 30.7us,
NEFF preamble ~6.7us, first-DMA completion ~13us, tail ~5.8us).
"""
import sys

if "/opt/trn_rl_repo" not in sys.path:
    sys.path.insert(0, "/opt/trn_rl_repo")

from contextlib import ExitStack

import numpy as np

import concourse.bacc as bacc
import concourse.mybir as mybir
import concourse.tile as tile
from concourse.bass_utils import run_bass_kernel_spmd

F32 = mybir.dt.float32
F32R = mybir.dt.float32r
F16 = mybir.dt.float16

B, S, D = 8, 2048, 768
KB = D // 128          # 6 contraction blocks
ST = S // 128          # 16 sequence tiles per core
CT = 4                 # sequence tiles per chunk
NCH = ST // CT         # 4 chunks
N_WARM = 48            # warm-up matmuls (N=128) during the DMA prologue
N_CORES = 8

MODE = "f16"


def build_program(mode=MODE):
    nc = bacc.Bacc()

    xt = nc.declare_dram_parameter("xt", [D, S], F16, isOutput=False)
    w = nc.declare_dram_parameter("w", [D, D], F16, isOutput=False)
    wacx = nc.declare_dram_parameter("wacx", [17, 1 + D], F16, isOutput=False)
    out = nc.declare_dram_parameter("out", [S, D], F16, isOutput=True)

    with tile.TileContext(nc) as tc, ExitStack() as ctx:
        const = ctx.enter_context(tc.tile_pool(name="const", bufs=1))
        wpool = ctx.enter_context(tc.tile_pool(name="wpool", bufs=1))
        xpool = ctx.enter_context(tc.tile_pool(name="xpool", bufs=3))
        opool = ctx.enter_context(tc.tile_pool(name="opool", bufs=2))
        pso = ctx.enter_context(tc.tile_pool(name="pso", bufs=1, space="PSUM"))

        # ---- warm tile memset on gpsimd; packed acx/wac goes FIRST on the
        # SP HWDGE ring (26KB) so its completion sem fires early -- the bias
        # matmuls head-of-line block the PE queue on it ----
        warm_sb = const.tile([128, 512], F16)
        nc.gpsimd.memset(warm_sb[:], 0.0)
        wacx_sb = const.tile([17, 1 + D], F16)
        nc.sync.dma_start(wacx_sb[:], wacx[:])
        bias_sb = const.tile([128, D], F32)

        # ---- prologue: 3 DMAs per HWDGE ring, sized so completions (FIFO
        # per ring) land just before each k-round needs them.  Ring A (SP)
        # carries all of x0 and finishes early, which hands ring B (ACT,
        # all of W) the full SDMA bandwidth for its tail.  7 total prologue
        # DMAs stay within the 8 completion-sem lanes (no issue blocking).
        wsb = wpool.tile([128, KB * D], F16)
        x_chunks = {}
        xs0 = xpool.tile([128, KB * CT * 128], F16, tag="xc", name="xc0")
        x_chunks[0] = xs0
        CW = CT * 128

        def _x0dma(k0, k1):
            if k1 - k0 == 1:
                nc.sync.dma_start(xs0[:, k0 * CW:k1 * CW],
                                  xt[k0 * 128:k1 * 128, 0:CW])
            else:
                nc.sync.dma_start(
                    xs0[:, k0 * CW:k1 * CW].rearrange(
                        "p (k s) -> p k s", k=k1 - k0),
                    xt[k0 * 128:k1 * 128, 0:CW].rearrange(
                        "(k p) s -> p k s", p=128))

        def _wdma(k0, k1):
            if k1 - k0 == 1:
                nc.scalar.dma_start(wsb[:, k0 * D:k1 * D],
                                    w[k0 * 128:k1 * 128, :])
            else:
                nc.scalar.dma_start(
                    wsb[:, k0 * D:k1 * D].rearrange(
                        "p (k n) -> p k n", k=k1 - k0),
                    w[k0 * 128:k1 * 128, :].rearrange(
                        "(k p) n -> p k n", p=128))

        _x0dma(0, 1)
        _wdma(0, 1)
        _x0dma(1, 3)
        _wdma(1, 3)
        _x0dma(3, 6)
        _wdma(3, 6)

        # ---- later x chunks on the SP ring (chunk 1 in halves) ----
        def xdma(c, halves=False):
            xs = xpool.tile([128, KB * CT * 128], F16, tag="xc", name=f"xc{c}")
            cs = slice(c * CT * 128, (c + 1) * CT * 128)
            if halves:
                half = CT * 128 // 2
                xs3d = xs[:].rearrange("p (k s) -> p k s", k=KB)
                for h in range(2):
                    hs = slice(c * CT * 128 + h * half,
                               c * CT * 128 + (h + 1) * half)
                    nc.sync.dma_start(
                        xs3d[:, :, h * half:(h + 1) * half],
                        xt[:, hs].rearrange("(k p) s -> p k s", p=128))
            else:
                nc.sync.dma_start(
                    xs[:].rearrange("p (k s) -> p k s", k=KB),
                    xt[:, cs].rearrange("(k p) s -> p k s", p=128))
            x_chunks[c] = xs

        xdma(1, halves=True)
        xdma(2)
        xdma(3)

        # ---- PE: warm-up matmuls into the p3 psum buffer ----
        warm_ps = pso.tile([128, D], F32, tag="p3", name="warm_ps")
        for i in range(N_WARM):
            nc.tensor.matmul(warm_ps[:, 0:128], warm_sb[:, 0:128],
                             warm_sb[:, 0:128], start=True, stop=True,
                             skip_group_check=True)

        # ---- PE: bias row = broadcast([ac|1]) @ [[Wa@Wv];[bt+ba@Wv+bv]] ----
        bias_ps = pso.tile([128, D], F32, tag="p3", name="bias_ps")
        acx_bc = wacx_sb[:, 0:1].broadcast_to([17, 128])
        nc.tensor.matmul(bias_ps[:, 0:512], acx_bc, wacx_sb[:, 1:513],
                         start=True, stop=True, skip_group_check=True)
        nc.tensor.matmul(bias_ps[:, 512:768], acx_bc, wacx_sb[:, 513:769],
                         start=True, stop=True, skip_group_check=True)
        nc.vector.tensor_copy(bias_sb[:], bias_ps[:])

        # ---- main loop ----
        def tile_matmuls(ps, xs, t, k):
            st, sp = (k == 0), (k == KB - 1)
            lhsT = xs[:, k * CT * 128 + t * 128:k * CT * 128 + (t + 1) * 128]
            nc.tensor.matmul(ps[:, 0:512], lhsT, wsb[:, k * D:k * D + 512],
                             start=st, stop=sp, skip_group_check=True)
            nc.tensor.matmul(ps[:, 512:768], lhsT,
                             wsb[:, k * D + 512:(k + 1) * D],
                             start=st, stop=sp, skip_group_check=True)

        def emit_chunk(c, k_major):
            xs = x_chunks.pop(c)
            osup = opool.tile([128, CT * D], F16, tag="osup", name=f"osup{c}")
            pss = [pso.tile([128, D], F32, tag=f"p{t}", name=f"ps{c}_{t}")
                   for t in range(CT)]
            if k_major:
                # zero-bridges (0*0 accumulated into the open group): keep the
                # PE issue stream gapless across feed micro-stalls -- any PE
                # idle gap resets the HAM warm-up window back to 1.2 GHz
                bridges = {0: 18, 1: 6, 2: 2}
                for k in range(KB):
                    for t in range(CT):
                        tile_matmuls(pss[t], xs, t, k)
                    for _ in range(bridges.get(k, 0)):
                        nc.tensor.matmul(pss[0][:, 0:128], warm_sb[:, 0:128],
                                         warm_sb[:, 0:128], start=False,
                                         stop=False, skip_group_check=True)
            else:
                for t in range(CT):
                    for k in range(KB):
                        tile_matmuls(pss[t], xs, t, k)
            for t in range(CT):
                last = (c == NCH - 1) and (t == CT - 1)
                if last:
                    # split the final eviction/store so the kernel-ending
                    # store (and its receipt) covers only 256 columns
                    ob = osup[:, t * D:(t + 1) * D]
                    i = c * CT + t
                    nc.vector.tensor_add(ob[:, 0:512], pss[t][:, 0:512],
                                         bias_sb[:, 0:512])
                    nc.scalar.dma_start(out[i * 128:(i + 1) * 128, 0:512],
                                        ob[:, 0:512])
                    nc.vector.tensor_add(ob[:, 512:768], pss[t][:, 512:768],
                                         bias_sb[:, 512:768])
                    nc.sync.dma_start(out[i * 128:(i + 1) * 128, 512:768],
                                      ob[:, 512:768])
                    continue
                nc.vector.tensor_add(osup[:, t * D:(t + 1) * D],
                                     pss[t][:], bias_sb[:])
                if c == NCH - 1:
                    i = c * CT + t
                    nc.scalar.dma_start(out[i * 128:(i + 1) * 128, :],
                                        osup[:, t * D:(t + 1) * D])
            if c != NCH - 1:
                nc.scalar.dma_start(
                    out[c * CT * 128:(c + 1) * CT * 128, :].rearrange(
                        "(j p) d -> p j d", p=128),
                    osup[:].rearrange("p (j d) -> p j d", j=CT))

        emit_chunk(0, k_major=True)
        emit_chunk(1, k_major=False)
        emit_chunk(2, k_major=False)
        emit_chunk(3, k_major=False)

    nc.compile()
    return nc


_PROGRAM_CACHE = {}


def _get_program(mode=None):
    if mode is None:
        mode = MODE
    if mode not in _PROGRAM_CACHE:
        _PROGRAM_CACHE[mode] = build_program(mode)
    return _PROGRAM_CACHE[mode]


def _build_in_maps(text_features, acoustic_features, Wt, bt, Wa, ba, Wv, bv,
                   **_unused):
    text_features = np.asarray(text_features, dtype=np.float32)
    acoustic_features = np.asarray(acoustic_features, dtype=np.float32)
    Wt = np.asarray(Wt, dtype=np.float32)
    bt = np.asarray(bt, dtype=np.float32)
    Wa = np.asarray(Wa, dtype=np.float32)
    ba = np.asarray(ba, dtype=np.float32)
    Wv = np.asarray(Wv, dtype=np.float32)
    bv = np.asarray(bv, dtype=np.float32)

    x16 = text_features.astype(np.float16)             # [B, S, D]
    w16 = np.ascontiguousarray(Wt.astype(np.float16))  # [D, D] k-major
    # weight-only folds (input-independent preprocessing)
    wa_fold = Wa.astype(np.float64) @ Wv.astype(np.float64)        # [16, D]
    c_fold = ba.astype(np.float64) @ Wv.astype(np.float64) + bv + bt

    in_maps = []
    for b in range(N_CORES):
        wacx = np.zeros((17, 1 + D), np.float16)
        wacx[0:16, 0] = acoustic_features[b]
        wacx[16, 0] = 1.0
        wacx[0:16, 1:] = wa_fold
        wacx[16, 1:] = c_fold
        in_maps.append(dict(
            xt=np.ascontiguousarray(x16[b].T),         # [D, S] fp16
            w=w16,
            wacx=wacx,
        ))
    return in_maps


def kernel(text_features, acoustic_features, Wt, bt, Wa, ba, Wq, bq, Wk, bk,
           Wv, bv, **_unused):
    nc = _get_program()
    in_maps = _build_in_maps(text_features, acoustic_features, Wt, bt, Wa, ba,
                             Wv, bv)
    res = run_bass_kernel_spmd(nc, in_maps, list(range(N_CORES))).results
    out = np.empty((B, S, D), dtype=np.float32)
    for b in range(N_CORES):
        out[b] = res[b]["out"].astype(np.float32)
    return out
